# revision 36
# baseline (speedup 1.0000x reference)
"""Trainium2 Bass kernel for nn_MixtureOfExperts (argmax-routed SwiGLU MoE).

Strategy (expert-parallel across 8 NeuronCores, bf16 matmuls):
  - Host computes router logits (fp64 matmul, tiny) and the argmax expert
    per token.  Top-2 logit gaps are >=1.7e-4 while fp32 rounding noise is
    ~1e-6, so routing is insensitive to arithmetic order.
  - Each core is assigned one expert and a fixed capacity of C=512 tokens
    (zero-padded).  Tokens beyond 512 for an overloaded expert (a few tens
    out of 4096) are computed on the host in fp32 — this keeps every core
    at exactly 512 tokens (perfect balance, and C=512 means every matmul
    streams full 512-row chunks with no partition-tile waste).
  - Each core computes the SwiGLU for its tokens only:
        h = silu(x @ gw) * (x @ uw);  y = h @ dw
    in bf16 (1 PE cycle/row, same rate as fp32r, half the HBM traffic).
  - Host scatters per-core outputs back to token positions.

Layout: x is shipped pre-transposed and k-major packed ([128, KD*C],
block k = x^T[k*128:(k+1)*128, :]) so the contraction dim D lands on SBUF
partitions; mm1 produces h^T [H, C] tiles which are exactly the stationary
operand layout needed for mm2 (contraction over H).  gate/up weights are
host-packed k-major and chunk-interleaved so each weight chunk is ONE
contiguous DMA.

Head scheduling (the measured bottleneck): trace analysis showed the PE's
first real matmul was gated on the slowest head DMA.  gpsimd DMAs go
through SWDGE (software descriptor generation, ~75GB/s + multi-us launch
latency), so no head DMA may use gpsimd.  x and the first two gate/up
h-slices are small separate DMAs (512KB each, sync/scalar hardware-DGE
queues) so mm1's first accumulation group starts as soon as ~1MB has
landed, not after the full 2.5MB prefix.  Only 6 PE-warmup matmuls
(clock p-state ramp 0.65->1.2->2.4GHz) cover the DMA window; the rest of
the ramp happens during real work.
"""

import numpy as np
import ml_dtypes

import concourse.mybir as mybir
import concourse.tile as tile
from concourse import bacc
from concourse.bass_utils import run_bass_kernel_spmd

B, T, D, E, H = 4, 1024, 1024, 8, 2048

BT = B * T
NCORES = 8
P = 128
KD = D // P   # k-tiles for mm1 (contraction over D)
KH = H // P   # k-tiles for mm2 (contraction over H)
C = 512       # per-core token capacity (matches PSUM bank free size)
F32 = mybir.dt.float32
BF16 = mybir.dt.bfloat16
NPBF16 = ml_dtypes.bfloat16

# gate/up weight chunks over H.  Early chunks are small so the first mm1
# groups are gated on small DMAs and the paced mid-chunks arrive with
# margin; later chunks are big (one contiguous DMA each, issue cost
# ~0.6us of sequencer time).
H_CHUNKS = [(0, 128), (128, 128), (256, 256), (512, 256), (768, 256),
            (1024, 512), (1536, 512)]
assert sum(c for _, c in H_CHUNKS) == H

# PE p-state warmup: small (64-row) matmuls bridge the window between
# engine start and the first real matmul's data with fine granularity.
# The early window (to ~20us) is fabric-supply-bound: both hw queues
# run at ~95GB/s under the 8-core head burst while mm1 wants ~148GB/s,
# so the PE inevitably idles ~2-4us before supply catches up.  A short
# warmup plus a mid-clock (re-ramping) start tracks the supply curve
# about as well as a hot start would — a longer bridge (measured with
# N_WARMUP=100) just converts supply stalls into later re-ramps and
# loses ~4us.  78 bridges to ~11.5-12.9us — just short of first-data on
# the slow-supply cores — trimming the idle+re-ramp without the hot-start
# pathology.
N_WARMUP = 78

_BUILD_CACHE = {}

# Optional kwargs forwarded to run_bass_kernel_spmd (test harness sets
# this to enable NTFF tracing; empty for normal use).
RUN_KWARGS = {}
LAST_RESULTS = None


def _build():
    """Build the per-core SPMD Bass kernel (capacity C tokens, bf16)."""
    nc = bacc.Bacc("TRN2", target_bir_lowering=False, debug=False)
    # k-major packed operands: block k of xt is x^T[k*128:(k+1)*128, :C]
    xt = nc.dram_tensor("xt", [P, KD * C], BF16, kind="ExternalInput")
    gu = nc.dram_tensor("gu", [P, KD * 2 * H], BF16, kind="ExternalInput")
    dw = nc.dram_tensor("dw", [P, KH * D], BF16, kind="ExternalInput")
    y = nc.dram_tensor("y", [C, D], BF16, kind="ExternalOutput")

    with tile.TileContext(nc) as tc:
        with (
            tc.tile_pool(name="xp", bufs=1) as xp,
            tc.tile_pool(name="dmp", bufs=1) as dmp,
            tc.tile_pool(name="hp", bufs=KH) as hp,
            tc.tile_pool(name="wp", bufs=1) as wp,
            tc.tile_pool(name="w2", bufs=2) as w2,
            tc.tile_pool(name="outp", bufs=3) as outp,
            tc.tile_pool(name="ps", bufs=8, space="PSUM") as ps,
        ):
            # Only SP (sync) and Activation (scalar) have hardware
            # descriptor generation — gpsimd falls back to slow SWDGE,
            # so it must never carry a deadline DMA.
            _eng_i = [0]

            def dma(dst, src, eng=None):
                engs = [nc.sync, nc.scalar]
                (eng or engs[_eng_i[0] % 2]).dma_start(dst, src)
                _eng_i[0] += 1

            # PE p-state warmup on a small memset tile.
            dmy = dmp.tile([P, P], BF16, tag="dmy")
            nc.gpsimd.memset(dmy[:], 0.0)
            pdmy = ps.tile([P, C], F32, tag="ps", name="pdmy")
            for _ in range(N_WARMUP):
                nc.tensor.matmul(pdmy[:, :64], lhsT=dmy[:, :P],
                                 rhs=dmy[:, :64], start=True, stop=True)

            # x in three separate tiles (k0-1, k2-3, k4-7) so mm1's first
            # k-steps are gated on a 256KB transfer, not the full 1MB.
            # (Separate tiles, not partial writes into one tile: Tile dep
            # tracking would gate readers on ALL writes to a shared tile.)
            xparts = [(0, 2), (2, 2), (4, 4)]
            xts = []
            for k0, kn in xparts:
                t = xp.tile([P, kn * C], BF16, tag=f"x{k0}", name=f"x{k0}")
                nc.sync.dma_start(t[:], xt[:, k0 * C:(k0 + kn) * C])
                xts.append((k0, kn, t))

            def x_k(k):
                for k0, kn, t in xts:
                    if k0 <= k < k0 + kn:
                        return t[:, (k - k0) * C:(k - k0 + 1) * C]
                raise AssertionError

            x_t = [x_k(k) for k in range(KD)]

            # gate/up chunks: ONE contiguous DMA per H-chunk (k-major
            # packed on host).  Within chunk ci at column base,
            # slice (k, hs, gate) = [base + k*2hcn + hs*P : +P]
            #       (k, hs, up)   = [base + k*2hcn + hcn + hs*P : +P]
            w_cols = []
            col = 0
            for hc0, hcn in H_CHUNKS:
                w_cols.append(col)
                col += KD * 2 * hcn

            # chunk 0 is two separate k-half tiles (256KB each) so pa(h0)
            # k0-3 is gated on 256KB, not 512KB.
            w_t = []
            w_t0 = []
            for ci, (hc0, hcn) in enumerate(H_CHUNKS):
                ncols = KD * 2 * hcn
                base = w_cols[ci]
                if ci == 0:
                    for half in range(2):
                        t = wp.tile([P, ncols // 2], BF16, tag=f"w0{half}",
                                    name=f"w0{half}")
                        nc.scalar.dma_start(
                            t[:], gu[:, base + half * ncols // 2:
                                      base + (half + 1) * ncols // 2])
                        w_t0.append(t)
                    w_t.append(None)
                    continue
                t = wp.tile([P, ncols], BF16, tag=f"w{ci}", name=f"w{ci}")
                if ci in (1, 2):
                    # all early weight chunks on scalar; sync carries ONLY
                    # x, so x's last piece (needed at t0+3.5us) lands ~1us
                    # earlier and the h0 stall shrinks.  ci2 lands ~1us
                    # later than on sync but is needed ~3us later still.
                    # (Splitting ci1/ci2 across both queues measured ~4us
                    # WORSE: the fabric is the binding constraint in the
                    # 8-20us window, so extra bytes on the "idle" sync
                    # queue slow the x pieces fabric-wide, and every
                    # reader then gates on both halves.)
                    nc.scalar.dma_start(t[:], gu[:, base:base + ncols])
                # ci >= 3: issued later, paced by mm1 progress (below)
                w_t.append(t)

            def w_ap(ci, k, hs, up):
                hcn = H_CHUNKS[ci][1]
                if ci == 0:
                    t = w_t0[k // (KD // 2)]
                    kk = k % (KD // 2)
                else:
                    t = w_t[ci]
                    kk = k
                off = kk * 2 * hcn + (hcn if up else 0) + hs * P
                return t[:, off:off + P]

            # down-proj weight tiles; DMAs paced by mm1 progress (below).
            dwt = [w2.tile([P, (KH // 2) * D], BF16, tag="w2",
                           name=f"dw{i}") for i in range(2)]

            # Just-in-time weight loads: with all 8 cores bursting their
            # full ~14MB at launch, aggregate demand exceeds the chip
            # fabric and the slowest core's head stretches.  Issuing the
            # later chunks from scalar's instruction stream between mm1
            # activations spreads the traffic over the mm1 window; each
            # chunk still lands well before it is consumed.
            def _load_chunk(ci):
                ncols = KD * 2 * H_CHUNKS[ci][1]
                nc.scalar.dma_start(w_t[ci][:],
                                    gu[:, w_cols[ci]:w_cols[ci] + ncols])

            # consumption starts: ci3 at h-tile 4, ci4 at 6, ci5 at 8,
            # ci6 at 12, dw0 at 16, dw1 at ~24-equivalent (mm2 halfway)
            def paced_loads(cn):
                if cn == 1:
                    _load_chunk(3)
                elif cn == 3:
                    _load_chunk(4)
                elif cn == 5:
                    _load_chunk(5)
                elif cn == 9:
                    _load_chunk(6)
                elif cn == 12:
                    nc.scalar.dma_start(dwt[0][:], dw[:, :(KH // 2) * D])
                elif cn == 14:
                    nc.scalar.dma_start(dwt[1][:], dw[:, (KH // 2) * D:])

            def dw_slice(k, nd0, ndn):
                t = dwt[k // (KH // 2)]
                base = (k % (KH // 2)) * D
                return t[:, base + nd0:base + nd0 + ndn]

            # ---- mm1: hT[j] = silu(gw.T x) * (uw.T x), tiled over H ----
            h_t = []
            for ci, (hc0, hcn) in enumerate(H_CHUNKS):
                for hs in range(hcn // P):
                    ht = hp.tile([P, C], BF16, tag="h")
                    pa = ps.tile([P, C], F32, tag="ps", name="pa")
                    pu = ps.tile([P, C], F32, tag="ps", name="pu")
                    for k in range(KD):
                        nc.tensor.matmul(
                            pa[:, :], lhsT=w_ap(ci, k, hs, False),
                            rhs=x_t[k],
                            start=(k == 0), stop=(k == KD - 1),
                        )
                        nc.tensor.matmul(
                            pu[:, :], lhsT=w_ap(ci, k, hs, True),
                            rhs=x_t[k],
                            start=(k == 0), stop=(k == KD - 1),
                        )
                    nc.scalar.activation(
                        ht[:, :], pa[:, :],
                        mybir.ActivationFunctionType.Silu,
                    )
                    nc.vector.tensor_mul(ht[:, :], ht[:, :], pu[:, :])
                    h_t.append(ht)
                    paced_loads(len(h_t))

            # ---- mm2: y = h @ dw, contraction over H ----
            for nd0 in range(0, D, C):
                for m in range(C // P):
                    py = ps.tile([P, C], F32, tag="ps", name="py")
                    for k in range(KH):
                        nc.tensor.matmul(
                            py[:, :],
                            lhsT=h_t[k][:, m * P:(m + 1) * P],
                            rhs=dw_slice(k, nd0, C),
                            start=(k == 0),
                            stop=(k == KH - 1),
                        )
                    # Drain PSUM -> SBUF (bf16 cast, half the DMA bytes)
                    # -> DRAM.  Ordinary tiles drain whole; the final tile
                    # drains in two halves with the last half partition-
                    # split across both queues, shortening the critical
                    # tail after the last matmul.
                    last = (nd0 == D - C) and (m == C // P - 1)
                    ot = outp.tile([P, C], BF16, tag="out")
                    if not last:
                        nc.vector.tensor_copy(ot[:, :], py[:, :])
                        dma(y[m * P:(m + 1) * P, nd0:nd0 + C], ot[:, :])
                    else:
                        # half-split drain with the two casts on separate
                        # engines (vector + scalar activation-copy) and
                        # the two DMAs on separate queues, so the critical
                        # tail after the last matmul is one 256-col copy
                        # + one 64KB DMA.  (DMAing straight from PSUM is
                        # not allowed — bass asserts src in SBUF/DRAM.)
                        hw = C // 2
                        nc.vector.tensor_copy(ot[:, :hw], py[:, :hw])
                        nc.scalar.activation(
                            ot[:, hw:], py[:, hw:],
                            mybir.ActivationFunctionType.Copy)
                        nc.sync.dma_start(y[m * P:(m + 1) * P, nd0:nd0 + hw],
                                          ot[:, :hw])
                        nc.scalar.dma_start(
                            y[m * P:(m + 1) * P, nd0 + hw:nd0 + C],
                            ot[:, hw:])

    nc.compile()
    return nc


def _get_kernel():
    if "k" not in _BUILD_CACHE:
        _BUILD_CACHE["k"] = _build()
    return _BUILD_CACHE["k"]


def _route(xf, gate_w):
    """argmax expert per token, computed in fp64 on host (negligible work)."""
    logits = xf.astype(np.float64) @ np.asarray(gate_w, np.float64).T
    return logits.argmax(axis=1)


def _bf16(a):
    return np.ascontiguousarray(np.asarray(a, np.float32)).astype(NPBF16)


def _pack_gu(gw_e, uw_e):
    """k-major chunk-interleaved [P, KD*2H]: chunk ci holds KD blocks of
    [gate[kP:(k+1)P, hc0:hc0+hcn] | up[...]]."""
    parts = []
    for hc0, hcn in H_CHUNKS:
        for k in range(KD):
            parts.append(gw_e[k * P:(k + 1) * P, hc0:hc0 + hcn])
            parts.append(uw_e[k * P:(k + 1) * P, hc0:hc0 + hcn])
    return np.ascontiguousarray(np.concatenate(parts, axis=1))


def _pack_k_major(a):
    """[R*P, N] -> [P, R*N] with block r = a[r*P:(r+1)*P, :]."""
    r = a.shape[0] // P
    return np.ascontiguousarray(
        a.reshape(r, P, a.shape[1]).transpose(1, 0, 2).reshape(P, -1))


def _silu_swiglu_host(xo, gw, uw, dwn):
    """fp32 reference path for host-computed overflow tokens."""
    a = xo @ gw
    u = xo @ uw
    h = u * (a / (1.0 + np.exp(-a)))
    return h @ dwn


def kernel(x, gate_w, gate_bank, up_bank, down_bank):
    global LAST_RESULTS
    x = np.asarray(x, np.float32)
    assert x.shape == (B, T, D)

    xf = np.ascontiguousarray(x.reshape(BT, D))
    sel = _route(xf, gate_w)
    idx = [np.nonzero(sel == e)[0] for e in range(E)]
    keep = [i[:C] for i in idx]
    over = [i[C:] for i in idx]

    nc = _get_kernel()

    gate_bank = np.asarray(gate_bank, np.float32)
    up_bank = np.asarray(up_bank, np.float32)
    down_bank = np.asarray(down_bank, np.float32)
    gb16 = _bf16(gate_bank)
    ub16 = _bf16(up_bank)
    db16 = _bf16(down_bank)
    x16 = _bf16(xf)

    in_maps = []
    for e in range(E):
        xe = np.zeros((D, C), NPBF16)
        n = len(keep[e])
        if n:
            xe[:, :n] = x16[keep[e]].T
        in_maps.append({
            "xt": _pack_k_major(xe),
            "gu": _pack_gu(gb16[e], ub16[e]),
            "dw": _pack_k_major(db16[e]),
        })

    res = run_bass_kernel_spmd(nc, in_maps, core_ids=list(range(NCORES)),
                               **RUN_KWARGS)
    LAST_RESULTS = res

    out = np.empty((BT, D), np.float32)
    for e in range(E):
        n = len(keep[e])
        if n:
            out[keep[e]] = res.results[e]["y"][:n].astype(np.float32)
        if len(over[e]):
            out[over[e]] = _silu_swiglu_host(
                xf[over[e]], gate_bank[e], up_bank[e], down_bank[e])
    return out.reshape(B, T, D)
